# revision 8
# baseline (speedup 1.0000x reference)
"""KernelNorm2d Trainium2 Bass kernel (fp16 I/O).

Problem: x [16, 64, 256, 256] f32. 2x2 windows (stride 2) over (H, W); per-window
statistics over (C, 2, 2) = 256 elements; out = (x - mean) / sqrt(var + eps),
same shape as x. Data-parallel over batch: 8 cores x 2 samples each.

The problem is HBM-bandwidth-bound; tolerance (2e-2) is far above fp16
round-off (~3e-4 measured end-to-end), so the host converts x to fp16 and the
kernel reads/writes fp16, halving HBM traffic vs f32.

Per-core layout: partition dim = window-row index i (nH = 128 exactly).
SBUF tile = [128(i), C=64, a=2, W=256] fp16 where a = row-within-window, so
partition i / free (c, a, w) holds x[b, c, 2*i+a, w]. For a fixed (i, c) the
two rows 2i, 2i+1 are contiguous in DRAM -> 1 KiB contiguous DMA runs.

Per sample b:
  - 1 load DMA (8 MB), 1 store DMA on a separate HWDGE ring (ACT-issued).
  - Window sums: one DVE tensor_reduce over [p, j(128), ca(128), b2(2)].
  - Sums of squares: ACT squares c-chunks into fp16 scratch, DVE reduces,
    partial sums accumulated.
  - Small [p,128] f32 ops -> inv_std and shift t = -mean*inv_std.
  - Normalize per window column j: x*inv + t, in place, split across
    DVE / ACT / GPSIMD.
"""

import os
import sys

for _p in ("/opt/trn_rl_repo", "/root/.axon_site/_ro/trn_rl_repo"):
    if os.path.isdir(_p) and _p not in sys.path:
        sys.path.append(_p)

import numpy as np

import concourse.bass as bass
import concourse.tile as tile
from concourse import bacc, mybir
from concourse.bass_utils import run_bass_kernel_spmd

# Problem constants (hardcoded per spec nn_KernelNorm2d_72164040507639)
B, C, H, W = 16, 64, 256, 256
N_CORES = 8
B_LOC = B // N_CORES          # samples per core
NH = H // 2                   # 128 window rows = partition dim
NJ = W // 2                   # 128 window cols
EPS = 1e-5
WIN = C * 4                   # 256 elements per window
CCH = 16                      # channels per square-scratch chunk

# normalize engine split over j (v=DVE, s=ACT, g=GPSIMD), weights ~ rates
NORM_PATTERN = "vgsvvsvv" * 2  # 10v / 4s / 2g per 16


def build_kernel(debug: bool = False) -> bass.Bass:
    nc = bacc.Bacc("TRN2", debug=debug)
    f16 = mybir.dt.float16
    f32 = mybir.dt.float32
    x = nc.dram_tensor("x", [B_LOC, C, NH, 2, W], f16, kind="ExternalInput")
    y = nc.dram_tensor("y", [B_LOC, C, NH, 2, W], f16, kind="ExternalOutput")

    with tile.TileContext(nc) as tc:
        with (
            tc.tile_pool(name="data", bufs=2) as data_pool,
            tc.tile_pool(name="stats", bufs=2) as stats_pool,
            tc.tile_pool(name="scratch", bufs=2) as scratch_pool,
            tc.tile_pool(name="singles", bufs=1) as singles,
        ):
            eps_tile = singles.tile([NH, 1], f32)
            nc.vector.memset(eps_tile, EPS)
            for b in range(B_LOC):
                xt = data_pool.tile([NH, C, 2, W], f16)
                # load: [i, c, a, w] <- x[b, c, i, a, w]; (a w) contiguous 1KiB
                nc.sync.dma_start(out=xt, in_=x[b].transpose([1, 0, 2, 3]))

                # 4D window views. The [p, j, b2, ca] order puts a strided dim
                # innermost: defeats the (uop-less) 16-bit packed DVE path so
                # the port-doubled 2x_2p mode can kick in instead.
                xt4 = xt.rearrange("p c a (j b2) -> p j (c a) b2", b2=2)
                xt5 = xt.rearrange("p c a (j b2) -> p j b2 (c a)", b2=2)

                # ---- window sums (DVE, one pass)
                s_sum = stats_pool.tile([NH, NJ], f32, tag="s_sum")
                nc.vector.tensor_reduce(
                    out=s_sum,
                    in_=xt5,
                    axis=mybir.AxisListType.XY,
                    op=mybir.AluOpType.add,
                )

                # ---- window sums of squares: ACT squares chunks, DVE reduces
                q_sum = stats_pool.tile([NH, NJ], f32, tag="q_sum")
                q_part = stats_pool.tile([NH, NJ], f32, tag="q_part")
                for ci in range(C // CCH):
                    cs = ci * CCH
                    x2 = scratch_pool.tile([NH, CCH, 2, W], f16, tag="x2")
                    nc.scalar.activation(
                        out=x2,
                        in_=xt[:, cs : cs + CCH],
                        func=mybir.ActivationFunctionType.Square,
                    )
                    x2v = x2.rearrange("p c a (j b2) -> p j b2 (c a)", b2=2)
                    tgt = q_sum if ci == 0 else q_part
                    nc.vector.tensor_reduce(
                        out=tgt,
                        in_=x2v,
                        axis=mybir.AxisListType.XY,
                        op=mybir.AluOpType.add,
                    )
                    if ci > 0:
                        nc.vector.tensor_add(out=q_sum, in0=q_sum, in1=q_part)

                # ---- stats: inv = 1/sqrt(E[x^2] - mean^2 + eps), t = -mean*inv
                nm = stats_pool.tile([NH, NJ], f32, tag="nm")
                var = stats_pool.tile([NH, NJ], f32, tag="var")
                nm2 = stats_pool.tile([NH, NJ], f32, tag="nm2")
                inv = stats_pool.tile([NH, NJ], f32, tag="inv")
                tsh = stats_pool.tile([NH, NJ], f32, tag="tsh")

                nc.vector.tensor_scalar_mul(out=nm, in0=s_sum, scalar1=-1.0 / WIN)
                nc.vector.tensor_mul(out=nm2, in0=nm, in1=nm)
                nc.vector.tensor_scalar_mul(out=var, in0=q_sum, scalar1=1.0 / WIN)
                nc.vector.tensor_tensor(
                    out=var, in0=var, in1=nm2, op=mybir.AluOpType.subtract
                )
                nc.scalar.activation(
                    out=var,
                    in_=var,
                    func=mybir.ActivationFunctionType.Sqrt,
                    bias=eps_tile,
                    scale=1.0,
                )
                nc.vector.reciprocal(out=inv, in_=var)
                nc.vector.tensor_mul(out=tsh, in0=nm, in1=inv)

                # ---- normalize in place: x*inv + t, DVE/ACT/GPSIMD split
                for j in range(NJ):
                    eng = NORM_PATTERN[j % len(NORM_PATTERN)]
                    if eng == "s":
                        win = xt4[:, j, :, :]
                        nc.scalar.activation(
                            out=win,
                            in_=win,
                            func=mybir.ActivationFunctionType.Identity,
                            bias=tsh[:, j : j + 1],
                            scale=inv[:, j : j + 1],
                        )
                    else:
                        win = xt5[:, j, :, :] if eng == "v" else xt4[:, j, :, :]
                        e = nc.vector if eng == "v" else nc.gpsimd
                        e.tensor_scalar(
                            out=win,
                            in0=win,
                            scalar1=inv[:, j : j + 1],
                            scalar2=tsh[:, j : j + 1],
                            op0=mybir.AluOpType.mult,
                            op1=mybir.AluOpType.add,
                        )

                # ---- store (ACT-issued HWDGE ring, separate FIFO from loads)
                nc.scalar.dma_start(out=y[b].transpose([1, 0, 2, 3]), in_=xt)
    nc.compile()
    return nc


_NC_CACHE = None
LAST_RESULTS = None


def _get_nc():
    global _NC_CACHE
    if _NC_CACHE is None:
        _NC_CACHE = build_kernel()
    return _NC_CACHE


def kernel(x: np.ndarray) -> np.ndarray:
    global LAST_RESULTS
    assert x.shape == (B, C, H, W), x.shape
    xh = np.ascontiguousarray(x, dtype=np.float16).reshape(B, C, NH, 2, W)
    nc = _get_nc()
    in_maps = [{"x": xh[k * B_LOC : (k + 1) * B_LOC]} for k in range(N_CORES)]
    kw = {}
    if os.environ.get("KERNEL_TRACE") == "1":
        kw["trace"] = True
        if os.environ.get("KERNEL_TRACE_DIR"):
            kw["tmpdir"] = os.environ["KERNEL_TRACE_DIR"]
    res = run_bass_kernel_spmd(nc, in_maps, core_ids=list(range(N_CORES)), **kw)
    LAST_RESULTS = res
    out = np.concatenate([r["y"] for r in res.results], axis=0)
    return out.astype(np.float32).reshape(B, C, H, W)


# revision 14
# speedup vs baseline: 1.5544x; 1.5544x over previous
"""KernelNorm2d Trainium2 Bass kernel (fp16 I/O).

Problem: x [16, 64, 256, 256] f32. 2x2 windows (stride 2) over (H, W); per-window
statistics over (C, 2, 2) = 256 elements; out = (x - mean) / sqrt(var + eps),
same shape as x. Data-parallel over batch: 8 cores x 2 samples each.

The problem is HBM-bandwidth-bound; tolerance (2e-2) is far above fp16
round-off (~3e-4 measured end-to-end), so the host converts x to fp16 and the
kernel reads/writes fp16, halving HBM traffic vs f32.

Per-core layout: partition dim = window-row index i (nH = 128 exactly).
SBUF tile = [128(i), C=64, a=2, W=256] fp16 where a = row-within-window, so
partition i / free (c, a, w) holds x[b, c, 2*i+a, w]. For a fixed (i, c) the
two rows 2i, 2i+1 are contiguous in DRAM -> 1 KiB contiguous DMA runs.

Per sample b:
  - 1 load DMA (8 MB), 1 store DMA on a separate HWDGE ring (ACT-issued).
  - Window sums: one DVE tensor_reduce over [p, j(128), ca(128), b2(2)].
  - Sums of squares: ACT squares c-chunks into fp16 scratch, DVE reduces,
    partial sums accumulated.
  - Small [p,128] f32 ops -> inv_std and shift t = -mean*inv_std.
  - Normalize per window column j: x*inv + t, in place, split across
    DVE / ACT / GPSIMD.
"""

import os
import sys

for _p in ("/opt/trn_rl_repo", "/root/.axon_site/_ro/trn_rl_repo"):
    if os.path.isdir(_p) and _p not in sys.path:
        sys.path.append(_p)

import numpy as np

import concourse.bass as bass
import concourse.tile as tile
from concourse import bacc, mybir
from concourse.bass_utils import run_bass_kernel_spmd

# Problem constants (hardcoded per spec nn_KernelNorm2d_72164040507639)
B, C, H, W = 16, 64, 256, 256
N_CORES = 8
B_LOC = B // N_CORES          # samples per core
NH = H // 2                   # 128 window rows = partition dim
NJ = W // 2                   # 128 window cols
EPS = 1e-5
WIN = C * 4                   # 256 elements per window
CCH = 16                      # channels per square-scratch chunk

# normalize engine split over j (v=DVE, s=ACT, g=GPSIMD), weights ~ rates
NORM_PATTERN = "vsgsgsvgsgsvgsgv"  # 4v / 6s / 6g per 16


def build_kernel(debug: bool = False) -> bass.Bass:
    nc = bacc.Bacc("TRN2", debug=debug)
    f16 = mybir.dt.float16
    f32 = mybir.dt.float32
    x = nc.dram_tensor("x", [B_LOC, C, NH, 2, W], f16, kind="ExternalInput")
    y = nc.dram_tensor("y", [B_LOC, C, NH, 2, W], f16, kind="ExternalOutput")

    with tile.TileContext(nc) as tc:
        with (
            tc.tile_pool(name="data", bufs=2) as data_pool,
            tc.tile_pool(name="stats", bufs=2) as stats_pool,
            tc.tile_pool(name="scratch", bufs=2) as scratch_pool,
            tc.tile_pool(name="singles", bufs=1) as singles,
        ):
            eps_tile = singles.tile([NH, 1], f32)
            nc.vector.memset(eps_tile, EPS)
            for b in range(B_LOC):
                xt = data_pool.tile([NH, C, 2, W], f16)
                # load: [i, c, a, w] <- x[b, c, i, a, w]; (a w) contiguous 1KiB
                nc.sync.dma_start(out=xt, in_=x[b].transpose([1, 0, 2, 3]))

                # 4D window view [p, j, ca, b2]: contiguous fp16 pairs innermost.
                # All-fp16 operands (including reduce outputs) unlock the DVE
                # 16-bit packed mode; any f32 operand forces 1x.
                xt4 = xt.rearrange("p c a (j b2) -> p j (c a) b2", b2=2)

                # ---- window sums (DVE, one pass); fp16 out is one final
                # rounding of an fp32-internal accumulation - fine at 2e-2 tol.
                s_sum = stats_pool.tile([NH, NJ], f16, tag="s_sum")
                with nc.allow_low_precision("fp32-internal accum, 2e-2 tol"):
                    nc.vector.tensor_reduce(
                        out=s_sum,
                        in_=xt4,
                        axis=mybir.AxisListType.XY,
                        op=mybir.AluOpType.add,
                    )

                # ---- window sums of squares: ACT squares chunks, DVE reduces
                q_sum = stats_pool.tile([NH, NJ], f16, tag="q_sum")
                q_part = stats_pool.tile([NH, NJ], f16, tag="q_part")
                for ci in range(C // CCH):
                    cs = ci * CCH
                    x2 = scratch_pool.tile([NH, CCH, 2, W], f16, tag="x2")
                    if ci % 2 == 0:
                        nc.scalar.activation(
                            out=x2,
                            in_=xt[:, cs : cs + CCH],
                            func=mybir.ActivationFunctionType.Square,
                        )
                    else:
                        nc.vector.tensor_mul(
                            out=x2,
                            in0=xt[:, cs : cs + CCH],
                            in1=xt[:, cs : cs + CCH],
                        )
                    x2v = x2.rearrange("p c a (j b2) -> p j (c a) b2", b2=2)
                    tgt = q_sum if ci == 0 else q_part
                    with nc.allow_low_precision("fp32-internal accum, 2e-2 tol"):
                        nc.vector.tensor_reduce(
                            out=tgt,
                            in_=x2v,
                            axis=mybir.AxisListType.XY,
                            op=mybir.AluOpType.add,
                        )
                    if ci > 0:
                        nc.vector.tensor_add(out=q_sum, in0=q_sum, in1=q_part)

                # ---- stats: inv = 1/sqrt(E[x^2] - mean^2 + eps), t = -mean*inv
                # (tiny [p,128] ops; f32 intermediates, fp16 final scalars)
                nm = stats_pool.tile([NH, NJ], f32, tag="nm")
                var = stats_pool.tile([NH, NJ], f32, tag="var")
                nm2 = stats_pool.tile([NH, NJ], f32, tag="nm2")
                inv = stats_pool.tile([NH, NJ], f32, tag="inv")
                tsh = stats_pool.tile([NH, NJ], f32, tag="tsh")

                nc.vector.tensor_scalar_mul(out=nm, in0=s_sum, scalar1=-1.0 / WIN)
                nc.vector.tensor_mul(out=nm2, in0=nm, in1=nm)
                nc.vector.tensor_scalar_mul(out=var, in0=q_sum, scalar1=1.0 / WIN)
                nc.vector.tensor_tensor(
                    out=var, in0=var, in1=nm2, op=mybir.AluOpType.subtract
                )
                nc.scalar.activation(
                    out=var,
                    in_=var,
                    func=mybir.ActivationFunctionType.Sqrt,
                    bias=eps_tile,
                    scale=1.0,
                )
                nc.vector.reciprocal(out=inv, in_=var)
                nc.vector.tensor_mul(out=tsh, in0=nm, in1=inv)

                # ---- normalize in place: x*inv + t, DVE/ACT/GPSIMD split
                for j in range(NJ):
                    win = xt4[:, j, :, :]
                    eng = NORM_PATTERN[j % len(NORM_PATTERN)]
                    if eng == "s":
                        nc.scalar.activation(
                            out=win,
                            in_=win,
                            func=mybir.ActivationFunctionType.Identity,
                            bias=tsh[:, j : j + 1],
                            scale=inv[:, j : j + 1],
                        )
                    else:
                        e = nc.vector if eng == "v" else nc.gpsimd
                        e.tensor_scalar(
                            out=win,
                            in0=win,
                            scalar1=inv[:, j : j + 1],
                            scalar2=tsh[:, j : j + 1],
                            op0=mybir.AluOpType.mult,
                            op1=mybir.AluOpType.add,
                        )

                # ---- store (ACT-issued HWDGE ring, separate FIFO from loads)
                nc.scalar.dma_start(out=y[b].transpose([1, 0, 2, 3]), in_=xt)
    nc.compile()
    return nc


_NC_CACHE = None
LAST_RESULTS = None


def _get_nc():
    global _NC_CACHE
    if _NC_CACHE is None:
        _NC_CACHE = build_kernel()
    return _NC_CACHE


def kernel(x: np.ndarray) -> np.ndarray:
    global LAST_RESULTS
    assert x.shape == (B, C, H, W), x.shape
    xh = np.ascontiguousarray(x, dtype=np.float16).reshape(B, C, NH, 2, W)
    nc = _get_nc()
    in_maps = [{"x": xh[k * B_LOC : (k + 1) * B_LOC]} for k in range(N_CORES)]
    kw = {}
    if os.environ.get("KERNEL_TRACE") == "1":
        kw["trace"] = True
        if os.environ.get("KERNEL_TRACE_DIR"):
            kw["tmpdir"] = os.environ["KERNEL_TRACE_DIR"]
    res = run_bass_kernel_spmd(nc, in_maps, core_ids=list(range(N_CORES)), **kw)
    LAST_RESULTS = res
    out = np.concatenate([r["y"] for r in res.results], axis=0)
    return out.astype(np.float32).reshape(B, C, H, W)


# revision 16
# speedup vs baseline: 1.6868x; 1.0852x over previous
"""KernelNorm2d Trainium2 Bass kernel (fp16 I/O).

Problem: x [16, 64, 256, 256] f32. 2x2 windows (stride 2) over (H, W); per-window
statistics over (C, 2, 2) = 256 elements; out = (x - mean) / sqrt(var + eps),
same shape as x. Data-parallel over batch: 8 cores x 2 samples each.

HBM-bandwidth-bound; tolerance (2e-2) is far above fp16 round-off (~3e-4
measured end-to-end), so the host converts to fp16 and the kernel reads/writes
fp16, halving HBM traffic vs f32.

Per-core layout: partition = window-row i (nH = 128). SBUF tile
[128(i), C=64, a=2, W=256] fp16; for fixed (i, c) rows 2i, 2i+1 are contiguous
in DRAM -> 1 KiB DMA runs (measured ~345 GB/s effective).

Engine facts (measured on HW):
  - DVE tensor_reduce: 1 elem/cyc with fp16 input regardless of out dtype
    (16-bit packed path has no uop -> falls to 1x); f32 input gets ~1.7x.
    -> squares are written as f32 by ACT, so the big sq-reduce runs ~1.7x.
  - Per-j normalize (per-partition scalars): DVE ~400ns / ACT ~550ns /
    GPSIMD ~760ns per 256-elem instruction; split across all three.
  - Work is software-pipelined: phase1 (load/reduce/stats) of both samples is
    emitted before the normalize+store phases, and stats/normalize are split
    per w-half so normalization starts as soon as half the stats are ready.
"""

import os
import sys

for _p in ("/opt/trn_rl_repo", "/root/.axon_site/_ro/trn_rl_repo"):
    if os.path.isdir(_p) and _p not in sys.path:
        sys.path.append(_p)

import numpy as np

import concourse.bass as bass
import concourse.tile as tile
from concourse import bacc, mybir
from concourse.bass_utils import run_bass_kernel_spmd

# Problem constants (hardcoded per spec nn_KernelNorm2d_72164040507639)
B, C, H, W = 16, 64, 256, 256
N_CORES = 8
B_LOC = B // N_CORES          # samples per core
NH = H // 2                   # 128 window rows = partition dim
NJ = W // 2                   # 128 window cols
NJH = NJ // 2                 # window cols per w-half
WH = W // 2
EPS = 1e-5
WIN = C * 4                   # 256 elements per window
CCH = 8                       # channels per square chunk

# normalize engine split per 64-j half (v=DVE, s=ACT, g=GPSIMD)
NV, NS = 15, 20  # rest -> GPSIMD


def _make_pattern(nv, ns, n=64):
    w = {"v": nv, "s": ns, "g": n - nv - ns}
    acc = {"v": 0.0, "s": 0.0, "g": 0.0}
    pat = []
    for k in range(n):
        best = max(w, key=lambda e: w[e] / n * (k + 1) - acc[e])
        acc[best] += 1
        pat.append(best)
    return "".join(pat)


NORM_PATTERN = _make_pattern(NV, NS)


def _norm_engine(nc, j):
    return NORM_PATTERN[j % 64]


def build_kernel(debug: bool = False) -> bass.Bass:
    nc = bacc.Bacc("TRN2", debug=debug)
    f16 = mybir.dt.float16
    f32 = mybir.dt.float32
    x = nc.dram_tensor("x", [B_LOC, C, NH, 2, W], f16, kind="ExternalInput")
    y = nc.dram_tensor("y", [B_LOC, C, NH, 2, W], f16, kind="ExternalOutput")

    with tile.TileContext(nc) as tc:
        with (
            tc.tile_pool(name="data", bufs=2) as data_pool,
            tc.tile_pool(name="stats", bufs=2) as stats_pool,
            tc.tile_pool(name="scratch", bufs=2) as scratch_pool,
            tc.tile_pool(name="singles", bufs=1) as singles,
        ):
            eps_tile = singles.tile([NH, 1], f32)
            nc.vector.memset(eps_tile, EPS)

            xts, invs, tshs = [], [], []
            # ---- phase 1 for both samples: load, sums, stats
            for b in range(B_LOC):
                xt = data_pool.tile([NH, C, 2, W], f16, tag="xt")
                xts.append(xt)
                for ch in range(2):
                    cs = ch * (C // 2)
                    nc.sync.dma_start(
                        out=xt[:, cs : cs + C // 2],
                        in_=x[b, cs : cs + C // 2].transpose([1, 0, 2, 3]),
                    )

                inv = stats_pool.tile([NH, NJ], f32, tag="inv")
                tsh = stats_pool.tile([NH, NJ], f32, tag="tsh")
                invs.append(inv)
                tshs.append(tsh)

                for h in range(2):
                    ws = h * WH
                    js = h * NJH
                    xh4 = xt[:, :, :, ws : ws + WH].rearrange(
                        "p c a (j b2) -> p j (c a) b2", b2=2
                    )
                    # window sums for this half (DVE 1x, unavoidable)
                    s_sum = stats_pool.tile([NH, NJH], f32, tag=f"s_sum{h}")
                    nc.vector.tensor_reduce(
                        out=s_sum,
                        in_=xh4,
                        axis=mybir.AxisListType.XY,
                        op=mybir.AluOpType.add,
                    )
                    # sums of squares: ACT writes f32 squares, DVE reduces ~1.7x
                    q_sum = stats_pool.tile([NH, NJH], f32, tag=f"q_sum{h}")
                    q_part = stats_pool.tile([NH, NJH], f32, tag=f"q_part{h}")
                    for ci in range(C // CCH):
                        cs = ci * CCH
                        x2 = scratch_pool.tile([NH, CCH, 2, WH], f32, tag=f"x2_{h}")
                        nc.scalar.activation(
                            out=x2,
                            in_=xt[:, cs : cs + CCH, :, ws : ws + WH],
                            func=mybir.ActivationFunctionType.Square,
                        )
                        x2v = x2.rearrange("p c a (j b2) -> p j (c a) b2", b2=2)
                        tgt = q_sum if ci == 0 else q_part
                        nc.vector.tensor_reduce(
                            out=tgt,
                            in_=x2v,
                            axis=mybir.AxisListType.XY,
                            op=mybir.AluOpType.add,
                        )
                        if ci > 0:
                            nc.vector.tensor_add(out=q_sum, in0=q_sum, in1=q_part)

                    # stats: inv = 1/sqrt(E[x^2]-mean^2+eps), t = -mean*inv
                    nm = stats_pool.tile([NH, NJH], f32, tag=f"nm{h}")
                    var = stats_pool.tile([NH, NJH], f32, tag=f"var{h}")
                    nm2 = stats_pool.tile([NH, NJH], f32, tag=f"nm2{h}")
                    ih = inv[:, js : js + NJH]
                    th = tsh[:, js : js + NJH]
                    nc.vector.tensor_scalar_mul(out=nm, in0=s_sum, scalar1=-1.0 / WIN)
                    nc.vector.tensor_mul(out=nm2, in0=nm, in1=nm)
                    nc.vector.tensor_scalar_mul(out=var, in0=q_sum, scalar1=1.0 / WIN)
                    nc.vector.tensor_tensor(
                        out=var, in0=var, in1=nm2, op=mybir.AluOpType.subtract
                    )
                    nc.scalar.activation(
                        out=var,
                        in_=var,
                        func=mybir.ActivationFunctionType.Sqrt,
                        bias=eps_tile,
                        scale=1.0,
                    )
                    nc.vector.reciprocal(out=ih, in_=var)
                    nc.vector.tensor_mul(out=th, in0=nm, in1=ih)

            # ---- phase 2: normalize in place + store
            for b in range(B_LOC):
                xt, inv, tsh = xts[b], invs[b], tshs[b]
                xt4 = xt.rearrange("p c a (j b2) -> p j (c a) b2", b2=2)
                for j in range(NJ):
                    win = xt4[:, j, :, :]
                    eng = _norm_engine(nc, j)
                    if eng == "s":
                        nc.scalar.activation(
                            out=win,
                            in_=win,
                            func=mybir.ActivationFunctionType.Identity,
                            bias=tsh[:, j : j + 1],
                            scale=inv[:, j : j + 1],
                        )
                    else:
                        e = nc.vector if eng == "v" else nc.gpsimd
                        e.tensor_scalar(
                            out=win,
                            in0=win,
                            scalar1=inv[:, j : j + 1],
                            scalar2=tsh[:, j : j + 1],
                            op0=mybir.AluOpType.mult,
                            op1=mybir.AluOpType.add,
                        )
                nc.scalar.dma_start(out=y[b].transpose([1, 0, 2, 3]), in_=xt)
    nc.compile()
    return nc


_NC_CACHE = None
LAST_RESULTS = None


def _get_nc():
    global _NC_CACHE
    if _NC_CACHE is None:
        _NC_CACHE = build_kernel()
    return _NC_CACHE


def kernel(x: np.ndarray) -> np.ndarray:
    global LAST_RESULTS
    assert x.shape == (B, C, H, W), x.shape
    xh = np.ascontiguousarray(x, dtype=np.float16).reshape(B, C, NH, 2, W)
    nc = _get_nc()
    in_maps = [{"x": xh[k * B_LOC : (k + 1) * B_LOC]} for k in range(N_CORES)]
    kw = {}
    if os.environ.get("KERNEL_TRACE") == "1":
        kw["trace"] = True
        if os.environ.get("KERNEL_TRACE_DIR"):
            kw["tmpdir"] = os.environ["KERNEL_TRACE_DIR"]
    res = run_bass_kernel_spmd(nc, in_maps, core_ids=list(range(N_CORES)), **kw)
    LAST_RESULTS = res
    out = np.concatenate([r["y"] for r in res.results], axis=0)
    return out.astype(np.float32).reshape(B, C, H, W)
